# revision 5
# baseline (speedup 1.0000x reference)
"""Trainium2 Bass kernel for BlockAttentionResidual.

Reference computation (fp32):
    K      = rmsnorm(V, w)                      # over d
    logits = einsum('d,lbtd->lbt', q, K)
    attn   = softmax(logits, axis=l)
    h      = einsum('lbt,lbtd->btd', attn, V)

v5 mapping (per NeuronCore, tokens = flattened (b,t) sharded 8 ways):
    - V relaid out ON THE HOST to [NT, P, L, D] fp16: each token-tile is
      ONE 2MB HWDGE DMA with contiguous 16KB partition lines.
    - ssq_l = sum_d V^2 and dot_l = sum_d V*qw: 16 reduce-class ops
      split by measured cost-model rates:
        ACT Square+accum     1225ns  (K_SQ_ACT of the squares)
        DVE TT-product (2x mode, 594ns/l, batched over l) followed by
            tensor_scalar+accum (4x mode, 327ns)  (rest)
        Pool TT-product (2127ns/l) + DVE ts+accum (K_DOT_POOL dots)
    - rsqrt via Newton from y0=1 on Pool (TT against const tiles).
    - softmax over l=8: max on DVE, Exp+accum on ACT (same table set as
      Square -> one table load), reciprocal on DVE.
    - h = sum_l e_l * V_l: diag(e_l) matmuls into PSUM on PE (16 x 512
      cols, kept dense to hold PE's fast p-state); diag blocks in ONE
      fused Pool TensorTensor [P, L, P].
    - 1/sum(e) folded into ACT PSUM->SBUF copy; h gathered in two
      [P, 8*D] SBUF halves, each stored as ONE 2MB contiguous DMA
      (h dram layout [2, P, 8*D]; host un-permutes).
"""

from contextlib import ExitStack

import numpy as np

import concourse.bass as bass
import concourse.mybir as mybir
import concourse.tile as tile
from concourse import bacc
from concourse.bass_utils import run_bass_kernel_spmd

NCORES = 8
L = 8
B = 4
T = 4096
D = 1024
BT = B * T
TOK = BT // NCORES  # tokens per core
P = 128
NT = TOK // P  # token tiles per core
NHALF = NT // 2
HALF = 512  # one PSUM bank of fp32 per matmul output
EPS = 1e-6
F32 = mybir.dt.float32
F16 = mybir.dt.float16

_CACHE: dict = {}

import os as _os

K_SQ_ACT = int(_os.environ.get("K_SQ_ACT", "6"))  # squares on ACT (rest DVE)
K_DOT_POOL = int(_os.environ.get("K_DOT_POOL", "2"))  # dot products premul on Pool
K_NEWTON = int(_os.environ.get("K_NEWTON", "2"))
K_VB = int(_os.environ.get("K_VB", "4"))  # V-tile bufs
K_PSUM = int(_os.environ.get("K_PSUM", "3"))
K_SB = int(_os.environ.get("K_SB", "6"))  # small-tile bufs
K_JB = int(_os.environ.get("K_JB", "3"))  # product scratch bufs
K_DIAG = _os.environ.get("K_DIAG", "fusedpool")  # fusedpool|fuseddve
K_ST_RING = _os.environ.get("K_ST_RING", "act")  # act|sync
K_HSMUL = _os.environ.get("K_HSMUL", "act")  # act|dve
K_DOT_MODE = _os.environ.get("K_DOT_MODE", "ttts")  # ttts|stt


def _build_nc(reps=1, sq_act=None, dot_pool=None, newton=None, vb=None,
              psum=None, sb=None, jb=None, diag=None, st_ring=None,
              hsmul=None, dot_mode=None):
    sq_act = K_SQ_ACT if sq_act is None else sq_act
    dot_pool = K_DOT_POOL if dot_pool is None else dot_pool
    newton = K_NEWTON if newton is None else newton
    vb = K_VB if vb is None else vb
    psum = K_PSUM if psum is None else psum
    sb = K_SB if sb is None else sb
    jb = K_JB if jb is None else jb
    diag = K_DIAG if diag is None else diag
    st_ring = K_ST_RING if st_ring is None else st_ring
    hsmul = K_HSMUL if hsmul is None else hsmul
    dot_mode = K_DOT_MODE if dot_mode is None else dot_mode
    A = mybir.ActivationFunctionType
    O = mybir.AluOpType
    X = mybir.AxisListType.X
    n_sq_dve = L - sq_act  # squares via DVE TT+ts
    n_dot_dve = L - dot_pool  # dots via DVE TT+ts (or stt)

    nc = bacc.Bacc(
        "TRN2",
        target_bir_lowering=False,
        debug=False,
        enable_asserts=False,
        num_devices=NCORES,
    )
    v_d = nc.dram_tensor("v", [NT, P, L, D], F16, kind="ExternalInput")
    qwb_d = nc.dram_tensor("qwb", [P, D], F16, kind="ExternalInput")
    id_d = nc.dram_tensor("ident", [P, P], F16, kind="ExternalInput")
    h_d = nc.dram_tensor("h", [2, P, NHALF * D], F16, kind="ExternalOutput")

    with tile.TileContext(nc) as tc, ExitStack() as ctx:
        cpool = ctx.enter_context(tc.tile_pool(name="const", bufs=1))
        vpool = ctx.enter_context(tc.tile_pool(name="vin", bufs=vb))
        spool = ctx.enter_context(tc.tile_pool(name="small", bufs=sb))
        jpool = ctx.enter_context(tc.tile_pool(name="scratch", bufs=jb))
        dpool = ctx.enter_context(tc.tile_pool(name="diag", bufs=3))
        ppool = ctx.enter_context(
            tc.tile_pool(name="psum", bufs=psum, space=bass.MemorySpace.PSUM)
        )

        qwb = cpool.tile([P, D], F16, tag="qwb")
        ident = cpool.tile([P, P], F16, tag="ident")
        nc.sync.dma_start(qwb[:], qwb_d[:])
        nc.sync.dma_start(ident[:], id_d[:])
        hhalf = [
            cpool.tile([P, NHALF * D], F16, tag=f"hh{k}", name=f"hh{k}")
            for k in range(2)
        ]

        # stride-0 sink for ACT Square primary output (only accum consumed)
        jact = jpool.tile([P, 1], F16, tag="jact", bufs=1)
        jact_out = jact.broadcast_to((P, D))
        # real packed sink for DVE tensor_scalar primary outputs: a
        # stride-0 out would break the 4x perf mode, so burn one [P, D]
        # scratch tile (WAW among DVE ts ops only - same engine, free)
        jts = jpool.tile([P, D], F16, tag="jts", bufs=1)

        zero_b = cpool.tile([P, 1], F32, tag="zero_b")
        nc.vector.memset(zero_b[:], 0.0)
        a0_b = cpool.tile([P, 1], F32, tag="a0_b")
        nc.vector.memset(a0_b[:], 1.5 - 0.5 * EPS)
        c1_b = cpool.tile([P, 1], F32, tag="c1_b")
        nc.vector.memset(c1_b[:], 0.5 / D)

        for rep_i in range(reps * NT):
            i = rep_i % NT
            k, j = divmod(i, NHALF)  # half index, slot within half

            vta = vpool.tile([P, L, D], F16, tag="vta", name="vta")
            nc.sync.dma_start(vta[:], v_d[i])

            ssq = spool.tile([P, L], F32, tag="ssq")
            dotv = spool.tile([P, L], F32, tag="dotv")

            # --- ssq: ACT squares ---
            for l in range(sq_act):
                nc.scalar.activation(
                    jact_out,
                    vta[:, l, :],
                    A.Square,
                    bias=zero_b[:],
                    accum_out=ssq[:, l : l + 1],
                )
            # --- ssq: DVE TT-square + ts-accum ---
            if n_sq_dve:
                jsq = jpool.tile([P, n_sq_dve, D], F16, tag="jsq", name="jsq")
                nc.vector.tensor_mul(
                    jsq[:], vta[:, sq_act:, :], vta[:, sq_act:, :]
                )
                for m in range(n_sq_dve):
                    nc.vector.tensor_scalar(
                        jts[:], jsq[:, m, :], 1.0, 0.0, O.mult, O.add,
                        accum_out=ssq[:, sq_act + m : sq_act + m + 1],
                    )
            # --- dots ---
            if dot_mode == "stt":
                for l in range(L):
                    jvec = jpool.tile([P, D], F16, tag="jvec", name="jvec", bufs=1)
                    nc.vector.scalar_tensor_tensor(
                        jvec[:], vta[:, l, :], 1.0, qwb[:], O.mult, O.mult,
                        accum_out=dotv[:, l : l + 1],
                    )
            else:
                if n_dot_dve:
                    jdd = jpool.tile([P, n_dot_dve, D], F16, tag="jdd", name="jdd")
                    nc.vector.tensor_tensor(
                        jdd[:],
                        vta[:, :n_dot_dve, :],
                        qwb[:].rearrange("p (o d) -> p o d", o=1).broadcast_to(
                            (P, n_dot_dve, D)
                        ),
                        O.mult,
                    )
                    for m in range(n_dot_dve):
                        nc.vector.tensor_scalar(
                            jts[:], jdd[:, m, :], 1.0, 0.0, O.mult, O.add,
                            accum_out=dotv[:, m : m + 1],
                        )
                if dot_pool:
                    jdp = jpool.tile([P, dot_pool, D], F16, tag="jdp", name="jdp")
                    nc.gpsimd.tensor_tensor(
                        jdp[:],
                        vta[:, n_dot_dve:, :],
                        qwb[:].rearrange("p (o d) -> p o d", o=1).broadcast_to(
                            (P, dot_pool, D)
                        ),
                        O.mult,
                    )
                    for m in range(dot_pool):
                        nc.vector.tensor_scalar(
                            jts[:], jdp[:, m, :], 1.0, 0.0, O.mult, O.add,
                            accum_out=dotv[:, n_dot_dve + m : n_dot_dve + m + 1],
                        )

            # inv = rsqrt(ssq/D + eps) by Newton from y0=1, all on Pool:
            #   y1 = A0 - s1, s1 = (0.5/D)*ssq;  y_{k+1} = y_k*(A0 - s1*y_k^2)
            tt = nc.gpsimd
            s1 = spool.tile([P, L], F32, tag="nwt_s1", name="nwt_s1")
            tt.tensor_mul(s1[:], ssq[:], c1_b.broadcast_to((P, L)))
            inv = spool.tile([P, L], F32, tag="inv")
            tt.tensor_tensor(inv[:], a0_b.broadcast_to((P, L)), s1[:], O.subtract)
            for _ in range(newton - 1):
                u = spool.tile([P, L], F32, tag="nwt_u", name="nwt_u")
                tt.tensor_mul(u[:], inv[:], inv[:])
                vh = spool.tile([P, L], F32, tag="nwt_v", name="nwt_vh")
                tt.tensor_mul(vh[:], s1[:], u[:])
                w = spool.tile([P, L], F32, tag="nwt_w", name="nwt_w")
                tt.tensor_tensor(w[:], a0_b.broadcast_to((P, L)), vh[:], O.subtract)
                inv2 = spool.tile([P, L], F32, tag="inv", name="inv2")
                tt.tensor_mul(inv2[:], inv[:], w[:])
                inv = inv2

            logits = spool.tile([P, L], F32, tag="logits")
            tt.tensor_mul(logits[:], dotv[:], inv[:])
            nm = spool.tile([P, 1], F32, tag="nm")
            nc.vector.tensor_reduce(nm[:], logits[:], X, O.max, negate=True)
            e = spool.tile([P, L], F32, tag="e")
            s = spool.tile([P, 1], F32, tag="s")
            nc.scalar.activation(e[:], logits[:], A.Exp, bias=nm[:], accum_out=s[:])
            r = spool.tile([P, 1], F32, tag="r")
            nc.vector.reciprocal(r[:], s[:])

            # h = sum_l e_l * V_l via diag(e_l) matmuls into PSUM;
            # 1/s applied in the PSUM->SBUF copy
            eng = nc.vector if diag == "fuseddve" else nc.gpsimd
            dga = dpool.tile([P, L, P], F16, tag="dga", name="dga")
            eng.tensor_tensor(
                dga[:],
                ident[:].rearrange("p (l c) -> p l c", l=1).broadcast_to(
                    (P, L, P)
                ),
                e[:].rearrange("p (l c) -> p l c", c=1).broadcast_to((P, L, P)),
                O.mult,
            )
            hp = ppool.tile([P, D], F32, tag="hp")
            for l in range(L):
                for h_ in range(2):
                    nc.tensor.matmul(
                        hp[:, h_ * HALF : (h_ + 1) * HALF],
                        dga[:, l, :],
                        vta[:, l, h_ * HALF : (h_ + 1) * HALF],
                        start=(l == 0),
                        stop=(l == L - 1),
                    )
            hs = hhalf[k][:, j * D : (j + 1) * D]
            if hsmul == "act":
                nc.scalar.mul(hs, hp[:], r[:])
            else:
                nc.vector.tensor_scalar_mul(hs, hp[:], r[:])
            if j == NHALF - 1:
                steng = nc.scalar if st_ring == "act" else nc.sync
                steng.dma_start(h_d[k], hhalf[k][:])

    nc.compile()
    return nc


def get_nc():
    if "nc" not in _CACHE:
        _CACHE["nc"] = _build_nc()
    return _CACHE["nc"]


def build_variant(**kw):
    return _build_nc(**kw)


def make_in_maps(blocks, query, norm_weight):
    qw = query.astype(np.float64) * norm_weight.astype(np.float64)
    qwb = np.ascontiguousarray(np.broadcast_to(qw, (P, D)).astype(np.float16))
    ident = np.eye(P, dtype=np.float32).astype(np.float16)
    vr = blocks.reshape(L, BT, D).astype(np.float16)
    in_maps = []
    for c in range(NCORES):
        vc = vr[:, c * TOK : (c + 1) * TOK, :]  # [L, TOK, D]
        v2 = np.ascontiguousarray(
            vc.reshape(L, NT, P, D).transpose(1, 2, 0, 3)
        )  # [NT, P, L, D]
        in_maps.append({"v": v2, "qwb": qwb, "ident": ident})
    return in_maps


def unpack_h(h_raw):
    # h_raw [2, P, NHALF*D] -> [TOK, D]
    return (
        h_raw.reshape(2, P, NHALF, D).transpose(0, 2, 1, 3).reshape(TOK, D)
    )


def kernel(blocks, query, norm_weight):
    import time

    blocks = np.asarray(blocks, dtype=np.float32)
    query = np.asarray(query, dtype=np.float32)
    norm_weight = np.asarray(norm_weight, dtype=np.float32)
    nc = get_nc()
    in_maps = make_in_maps(blocks, query, norm_weight)
    last_exc = None
    for attempt in range(3):
        try:
            res = run_bass_kernel_spmd(nc, in_maps, core_ids=list(range(NCORES)))
            break
        except Exception as exc:  # transient device-wedge after a prior crash
            last_exc = exc
            time.sleep(45)
    else:
        raise last_exc
    h = np.concatenate(
        [
            unpack_h(np.asarray(res.results[c]["h"]).astype(np.float32))
            for c in range(NCORES)
        ],
        axis=0,
    )
    return h.reshape(B, T, D)


# revision 10
# speedup vs baseline: 2.2658x; 2.2658x over previous
"""Trainium2 Bass kernel for BlockAttentionResidual.

Reference computation (fp32):
    K      = rmsnorm(V, w)                      # over d
    logits = einsum('d,lbtd->lbt', q, K)
    attn   = softmax(logits, axis=l)
    h      = einsum('lbt,lbtd->btd', attn, V)

v5 mapping (per NeuronCore, tokens = flattened (b,t) sharded 8 ways):
    - V relaid out ON THE HOST to [NT, P, L, D] fp16: each token-tile is
      ONE 2MB HWDGE DMA with contiguous 16KB partition lines.
    - ssq_l = sum_d V^2 and dot_l = sum_d V*qw: 16 reduce-class ops
      split by measured cost-model rates:
        ACT Square+accum     1225ns  (K_SQ_ACT of the squares)
        DVE TT-product (2x mode, 594ns/l, batched over l) followed by
            tensor_scalar+accum (4x mode, 327ns)  (rest)
        Pool TT-product (2127ns/l) + DVE ts+accum (K_DOT_POOL dots)
    - rsqrt via Newton from y0=1 on Pool (TT against const tiles).
    - softmax over l=8: max on DVE, Exp+accum on ACT (same table set as
      Square -> one table load), reciprocal on DVE.
    - h = sum_l e_l * V_l: diag(e_l) matmuls into PSUM on PE (16 x 512
      cols, kept dense to hold PE's fast p-state); diag blocks in ONE
      fused Pool TensorTensor [P, L, P].
    - 1/sum(e) folded into ACT PSUM->SBUF copy; h gathered in two
      [P, 8*D] SBUF halves, each stored as ONE 2MB contiguous DMA
      (h dram layout [2, P, 8*D]; host un-permutes).
"""

from contextlib import ExitStack

import numpy as np

import concourse.bass as bass
import concourse.mybir as mybir
import concourse.tile as tile
from concourse import bacc
from concourse.bass_utils import run_bass_kernel_spmd

NCORES = 8
L = 8
B = 4
T = 4096
D = 1024
BT = B * T
TOK = BT // NCORES  # tokens per core
P = 128
NT = TOK // P  # token tiles per core
NHALF = NT // 2
HALF = 512  # one PSUM bank of fp32 per matmul output
EPS = 1e-6
F32 = mybir.dt.float32
F16 = mybir.dt.float16

_CACHE: dict = {}

import os as _os

K_SQ_ACT = int(_os.environ.get("K_SQ_ACT", "6"))  # squares on ACT (rest DVE)
K_DOT_POOL = int(_os.environ.get("K_DOT_POOL", "2"))  # dot products premul on Pool
K_NEWTON = int(_os.environ.get("K_NEWTON", "2"))
K_VB = int(_os.environ.get("K_VB", "4"))  # V-tile bufs
K_PSUM = int(_os.environ.get("K_PSUM", "3"))
K_SB = int(_os.environ.get("K_SB", "6"))  # small-tile bufs
K_JB = int(_os.environ.get("K_JB", "3"))  # product scratch bufs
K_DIAG = _os.environ.get("K_DIAG", "fusedpool")  # fusedpool|fuseddve
K_ST_RING = _os.environ.get("K_ST_RING", "act")  # act|sync
K_HSMUL = _os.environ.get("K_HSMUL", "act")  # act|dve
K_DOT_MODE = _os.environ.get("K_DOT_MODE", "ttts")  # ttts|stt


def _build_nc(reps=1, sq_act=None, dot_pool=None, newton=None, vb=None,
              psum=None, sb=None, jb=None, diag=None, st_ring=None,
              hsmul=None, dot_mode=None, mode="full", ld_ring="sync",
              sq_pool=0, store="half", small_eng="pool"):
    sq_act = K_SQ_ACT if sq_act is None else sq_act
    dot_pool = K_DOT_POOL if dot_pool is None else dot_pool
    newton = K_NEWTON if newton is None else newton
    vb = K_VB if vb is None else vb
    psum = K_PSUM if psum is None else psum
    sb = K_SB if sb is None else sb
    jb = K_JB if jb is None else jb
    diag = K_DIAG if diag is None else diag
    st_ring = K_ST_RING if st_ring is None else st_ring
    hsmul = K_HSMUL if hsmul is None else hsmul
    dot_mode = K_DOT_MODE if dot_mode is None else dot_mode
    A = mybir.ActivationFunctionType
    O = mybir.AluOpType
    X = mybir.AxisListType.X
    n_sq_dve = L - sq_act - sq_pool  # squares via DVE TT+ts
    n_dot_dve = L - dot_pool  # dots via DVE TT+ts (or stt)

    nc = bacc.Bacc(
        "TRN2",
        target_bir_lowering=False,
        debug=False,
        enable_asserts=False,
        num_devices=NCORES,
    )
    v_d = nc.dram_tensor("v", [NT, P, L, D], F16, kind="ExternalInput")
    qwb_d = nc.dram_tensor("qwb", [P, D], F16, kind="ExternalInput")
    id_d = nc.dram_tensor("ident", [P, P], F16, kind="ExternalInput")
    h_d = nc.dram_tensor("h", [2, P, NHALF * D], F16, kind="ExternalOutput")

    with tile.TileContext(nc) as tc, ExitStack() as ctx:
        cpool = ctx.enter_context(tc.tile_pool(name="const", bufs=1))
        vpool = ctx.enter_context(tc.tile_pool(name="vin", bufs=vb))
        spool = ctx.enter_context(tc.tile_pool(name="small", bufs=sb))
        jpool = ctx.enter_context(tc.tile_pool(name="scratch", bufs=jb))
        dpool = ctx.enter_context(tc.tile_pool(name="diag", bufs=3))
        ppool = ctx.enter_context(
            tc.tile_pool(name="psum", bufs=psum, space=bass.MemorySpace.PSUM)
        )

        qwb = cpool.tile([P, D], F16, tag="qwb")
        ident = cpool.tile([P, P], F16, tag="ident")
        nc.sync.dma_start(qwb[:], qwb_d[:])
        nc.sync.dma_start(ident[:], id_d[:])
        hhalf = [
            cpool.tile([P, NHALF * D], F16, tag=f"hh{k}", name=f"hh{k}")
            for k in range(2)
        ]

        # stride-0 sink for ACT Square primary output (only accum consumed)
        jact = jpool.tile([P, 1], F16, tag="jact", bufs=1)
        jact_out = jact.broadcast_to((P, D))
        # real packed sink for DVE tensor_scalar primary outputs: a
        # stride-0 out would break the 4x perf mode, so burn one [P, D]
        # scratch tile (WAW among DVE ts ops only - same engine, free)
        jts = jpool.tile([P, D], F16, tag="jts", bufs=1)

        zero_b = cpool.tile([P, 1], F32, tag="zero_b")
        nc.vector.memset(zero_b[:], 0.0)
        a0_b = cpool.tile([P, 1], F32, tag="a0_b")
        nc.vector.memset(a0_b[:], 1.5 - 0.5 * EPS)
        c1_b = cpool.tile([P, 1], F32, tag="c1_b")
        nc.vector.memset(c1_b[:], 0.5 / D)

        for rep_i in range(reps * NT):
            i = rep_i % NT
            k, j = divmod(i, NHALF)  # half index, slot within half

            vta = vpool.tile([P, L, D], F16, tag="vta", name="vta")
            ldeng = nc.scalar if (ld_ring == "alt" and i % 2) else nc.sync
            ldeng.dma_start(vta[:], v_d[i])

            if mode == "dmaonly":
                # loads + a token DVE op + stores: HBM roofline probe
                hs0 = hhalf[k][:, j * D : (j + 1) * D]
                nc.vector.tensor_copy(hs0, vta[:, 0, :])
                if j == NHALF - 1:
                    steng = nc.scalar if st_ring == "act" else nc.sync
                    steng.dma_start(h_d[k], hhalf[k][:])
                continue

            ssq = spool.tile([P, L], F32, tag="ssq")
            dotv = spool.tile([P, L], F32, tag="dotv")

            # --- ssq: ACT squares ---
            for l in range(sq_act):
                nc.scalar.activation(
                    jact_out,
                    vta[:, l, :],
                    A.Square,
                    bias=zero_b[:],
                    accum_out=ssq[:, l : l + 1],
                )
            # --- ssq: Pool premult + DVE ts-accum ---
            if sq_pool:
                jsp = jpool.tile([P, sq_pool, D], F16, tag="jsp", name="jsp")
                nc.gpsimd.tensor_tensor(
                    jsp[:],
                    vta[:, sq_act : sq_act + sq_pool, :],
                    vta[:, sq_act : sq_act + sq_pool, :],
                    O.mult,
                )
                for m in range(sq_pool):
                    nc.vector.tensor_scalar(
                        jts[:], jsp[:, m, :], 1.0, 0.0, O.mult, O.add,
                        accum_out=ssq[:, sq_act + m : sq_act + m + 1],
                    )
            # --- ssq: DVE TT-square + ts-accum ---
            if n_sq_dve:
                sq0 = sq_act + sq_pool
                jsq = jpool.tile([P, n_sq_dve, D], F16, tag="jsq", name="jsq")
                nc.vector.tensor_mul(
                    jsq[:], vta[:, sq0:, :], vta[:, sq0:, :]
                )
                for m in range(n_sq_dve):
                    nc.vector.tensor_scalar(
                        jts[:], jsq[:, m, :], 1.0, 0.0, O.mult, O.add,
                        accum_out=ssq[:, sq0 + m : sq0 + m + 1],
                    )
            # --- dots ---
            if dot_mode == "stt":
                for l in range(n_dot_dve):
                    jvec = jpool.tile([P, D], F16, tag="jvec", name="jvec", bufs=1)
                    nc.vector.scalar_tensor_tensor(
                        jvec[:], vta[:, l, :], 1.0, qwb[:], O.mult, O.mult,
                        accum_out=dotv[:, l : l + 1],
                    )
                if dot_pool:
                    jdp = jpool.tile([P, dot_pool, D], F16, tag="jdp", name="jdp")
                    nc.gpsimd.tensor_tensor(
                        jdp[:],
                        vta[:, n_dot_dve:, :],
                        qwb[:].rearrange("p (o d) -> p o d", o=1).broadcast_to(
                            (P, dot_pool, D)
                        ),
                        O.mult,
                    )
                    for m in range(dot_pool):
                        nc.vector.tensor_scalar(
                            jts[:], jdp[:, m, :], 1.0, 0.0, O.mult, O.add,
                            accum_out=dotv[:, n_dot_dve + m : n_dot_dve + m + 1],
                        )
            else:
                if n_dot_dve:
                    jdd = jpool.tile([P, n_dot_dve, D], F16, tag="jdd", name="jdd")
                    nc.vector.tensor_tensor(
                        jdd[:],
                        vta[:, :n_dot_dve, :],
                        qwb[:].rearrange("p (o d) -> p o d", o=1).broadcast_to(
                            (P, n_dot_dve, D)
                        ),
                        O.mult,
                    )
                    for m in range(n_dot_dve):
                        nc.vector.tensor_scalar(
                            jts[:], jdd[:, m, :], 1.0, 0.0, O.mult, O.add,
                            accum_out=dotv[:, m : m + 1],
                        )
                if dot_pool:
                    jdp = jpool.tile([P, dot_pool, D], F16, tag="jdp", name="jdp")
                    nc.gpsimd.tensor_tensor(
                        jdp[:],
                        vta[:, n_dot_dve:, :],
                        qwb[:].rearrange("p (o d) -> p o d", o=1).broadcast_to(
                            (P, dot_pool, D)
                        ),
                        O.mult,
                    )
                    for m in range(dot_pool):
                        nc.vector.tensor_scalar(
                            jts[:], jdp[:, m, :], 1.0, 0.0, O.mult, O.add,
                            accum_out=dotv[:, n_dot_dve + m : n_dot_dve + m + 1],
                        )

            # inv = rsqrt(ssq/D + eps) by Newton from y0=1:
            #   y1 = A0 - s1, s1 = (0.5/D)*ssq;  y_{k+1} = y_k*(A0 - s1*y_k^2)
            tt = nc.gpsimd if small_eng == "pool" else nc.vector
            s1 = spool.tile([P, L], F32, tag="nwt_s1", name="nwt_s1")
            tt.tensor_mul(s1[:], ssq[:], c1_b.broadcast_to((P, L)))
            inv = spool.tile([P, L], F32, tag="inv")
            tt.tensor_tensor(inv[:], a0_b.broadcast_to((P, L)), s1[:], O.subtract)
            for _ in range(newton - 1):
                u = spool.tile([P, L], F32, tag="nwt_u", name="nwt_u")
                tt.tensor_mul(u[:], inv[:], inv[:])
                vh = spool.tile([P, L], F32, tag="nwt_v", name="nwt_vh")
                tt.tensor_mul(vh[:], s1[:], u[:])
                w = spool.tile([P, L], F32, tag="nwt_w", name="nwt_w")
                tt.tensor_tensor(w[:], a0_b.broadcast_to((P, L)), vh[:], O.subtract)
                inv2 = spool.tile([P, L], F32, tag="inv", name="inv2")
                tt.tensor_mul(inv2[:], inv[:], w[:])
                inv = inv2

            logits = spool.tile([P, L], F32, tag="logits")
            tt.tensor_mul(logits[:], dotv[:], inv[:])
            nm = spool.tile([P, 1], F32, tag="nm")
            nc.vector.tensor_reduce(nm[:], logits[:], X, O.max, negate=True)
            e = spool.tile([P, L], F32, tag="e")
            s = spool.tile([P, 1], F32, tag="s")
            nc.scalar.activation(e[:], logits[:], A.Exp, bias=nm[:], accum_out=s[:])
            r = spool.tile([P, 1], F32, tag="r")
            nc.vector.reciprocal(r[:], s[:])

            # h = sum_l e_l * V_l via diag(e_l) matmuls into PSUM;
            # 1/s applied in the PSUM->SBUF copy
            eng = nc.vector if diag == "fuseddve" else nc.gpsimd
            dga = dpool.tile([P, L, P], F16, tag="dga", name="dga")
            eng.tensor_tensor(
                dga[:],
                ident[:].rearrange("p (l c) -> p l c", l=1).broadcast_to(
                    (P, L, P)
                ),
                e[:].rearrange("p (l c) -> p l c", c=1).broadcast_to((P, L, P)),
                O.mult,
            )
            hp = ppool.tile([P, D], F32, tag="hp")
            for l in range(L):
                for h_ in range(2):
                    nc.tensor.matmul(
                        hp[:, h_ * HALF : (h_ + 1) * HALF],
                        dga[:, l, :],
                        vta[:, l, h_ * HALF : (h_ + 1) * HALF],
                        start=(l == 0),
                        stop=(l == L - 1),
                    )
            hs = hhalf[k][:, j * D : (j + 1) * D]
            if hsmul == "act":
                nc.scalar.mul(hs, hp[:], r[:])
            else:
                nc.vector.tensor_scalar_mul(hs, hp[:], r[:])
            steng = nc.scalar if st_ring == "act" else nc.sync
            if store == "tile":
                steng.dma_start(h_d[k, :, j * D : (j + 1) * D], hs)
            elif store == "tail1":
                # bulk store of the first NHALF-1 slots mid-stream, then a
                # small final store so the kernel tail is only ~256KB
                if j == NHALF - 2:
                    steng.dma_start(
                        h_d[k, :, : (NHALF - 1) * D],
                        hhalf[k][:, : (NHALF - 1) * D],
                    )
                elif j == NHALF - 1:
                    steng.dma_start(h_d[k, :, (NHALF - 1) * D :], hs)
            else:
                if j == NHALF - 1:
                    steng.dma_start(h_d[k], hhalf[k][:])

    nc.compile()
    return nc


def get_nc():
    if "nc" not in _CACHE:
        _CACHE["nc"] = _build_nc()
    return _CACHE["nc"]


def build_variant(**kw):
    return _build_nc(**kw)


def make_in_maps(blocks, query, norm_weight):
    qw = query.astype(np.float64) * norm_weight.astype(np.float64)
    qwb = np.ascontiguousarray(np.broadcast_to(qw, (P, D)).astype(np.float16))
    ident = np.eye(P, dtype=np.float32).astype(np.float16)
    vr = blocks.reshape(L, BT, D).astype(np.float16)
    in_maps = []
    for c in range(NCORES):
        vc = vr[:, c * TOK : (c + 1) * TOK, :]  # [L, TOK, D]
        v2 = np.ascontiguousarray(
            vc.reshape(L, NT, P, D).transpose(1, 2, 0, 3)
        )  # [NT, P, L, D]
        in_maps.append({"v": v2, "qwb": qwb, "ident": ident})
    return in_maps


def unpack_h(h_raw):
    # h_raw [2, P, NHALF*D] -> [TOK, D]
    return (
        h_raw.reshape(2, P, NHALF, D).transpose(0, 2, 1, 3).reshape(TOK, D)
    )


def kernel(blocks, query, norm_weight):
    import time

    blocks = np.asarray(blocks, dtype=np.float32)
    query = np.asarray(query, dtype=np.float32)
    norm_weight = np.asarray(norm_weight, dtype=np.float32)
    nc = get_nc()
    in_maps = make_in_maps(blocks, query, norm_weight)
    last_exc = None
    for attempt in range(3):
        try:
            res = run_bass_kernel_spmd(nc, in_maps, core_ids=list(range(NCORES)))
            break
        except Exception as exc:  # transient device-wedge after a prior crash
            last_exc = exc
            time.sleep(45)
    else:
        raise last_exc
    h = np.concatenate(
        [
            unpack_h(np.asarray(res.results[c]["h"]).astype(np.float32))
            for c in range(NCORES)
        ],
        axis=0,
    )
    return h.reshape(B, T, D)
